# revision 12
# baseline (speedup 1.0000x reference)
"""Neighbor aggregation (GNN message passing) on 8 Trainium2 cores.

out[b, u] = sum_{edges (u, v) in batch b} w_e * H[b, v]    (messages are 16x16 blocks)

Sharding: core (b, h) handles batch b's edges whose destination falls in
dst-half h (h=0: nodes [0, 25088), h=1: [25088, 50048)).  Outputs are disjoint
-> no cross-core reduction.  Within a core, edges are bucketed by 128-node dst
block and by src half (so gather indices fit in int16), padded to a uniform
number of 128-edge groups per bucket.  Device loop per dst block:
  dma_gather 128-row groups of H (bf16, SWDGE) -- calls balanced across the 4
  SWDGE queues so descriptor generation overlaps across Q7 CPU pairs
  one-hot weight matrices W[e, d] = w_e * (d == dstloc_e) are precomputed on
  the host and streamed per block (one dma_start per block, no DVE work)
  PSUM += W.T @ Hgather                                    (bf16 matmul)
  copy PSUM -> SBUF -> DRAM out rows (fp32).
"""

import sys

sys.path.insert(0, "/opt/trn_rl_repo")

import numpy as np
import ml_dtypes

import concourse.bacc as bacc
import concourse.tile as tile
from concourse import mybir
from concourse.bass_utils import run_bass_kernel_spmd

B = 4
N_NODES = 50000
HS = 16
C = HS * HS          # 256 values per message row
P = 128
E = 800000
NBLK = 196           # dst blocks per core (196*128 = 25088 rows of output)
HALF0 = NBLK * P     # dst boundary between the two cores of a batch
SRC_SPLIT = 25000    # src half boundary; local indices stay < 32768 (int16)
NSEG = NBLK * 2      # (block, src-half) buckets per core
N_CORES = 8
NQ = 4               # SWDGE queues (gather descriptor-gen pipelines)

_prog_cache: dict[int, object] = {}
_last_in_maps: list | None = None


def _build_program(gh: int):
    """Bass program for all 8 cores; gh = 128-edge groups per (block, src-half)."""
    ngrp = NSEG * gh             # one-hot groups per core
    idx_cols = NSEG * gh * 8     # int16 idx columns (16 idx per column)

    nc = bacc.Bacc("TRN2", target_bir_lowering=False, debug=False,
                   num_swdge_queues=NQ, dynamic_dma_scratch_size=49152)
    h_d = nc.dram_tensor("h", (N_NODES, C), mybir.dt.bfloat16, kind="ExternalInput")
    idx_d = nc.dram_tensor("idx", (P, idx_cols), mybir.dt.int16, kind="ExternalInput")
    w_d = nc.dram_tensor("w", (P, ngrp * P), mybir.dt.bfloat16, kind="ExternalInput")
    out_d = nc.dram_tensor("out", (NBLK * P, C), mybir.dt.bfloat16, kind="ExternalOutput")

    h_ap = h_d.ap()
    src_half_aps = (h_ap[0:SRC_SPLIT, :], h_ap[SRC_SPLIT:N_NODES, :])

    nstream = NBLK * gh          # gather groups per src-half stream
    ncall = -(-nstream // 8)     # 1024-idx calls per half
    qctr = 0

    with tile.TileContext(nc) as tc:
        with tc.tile_pool(name="const", bufs=1) as cpool, \
             tc.tile_pool(name="gat", bufs=12) as gpool, \
             tc.tile_pool(name="wtile", bufs=4) as wpool, \
             tc.tile_pool(name="otile", bufs=4) as opool, \
             tc.tile_pool(name="psum", bufs=8, space="PSUM") as ppool:
            idx_t = cpool.tile([P, idx_cols], mybir.dt.int16)
            nc.sync.dma_start(out=idx_t[:], in_=idx_d.ap())

            # each src-half is one contiguous stream of 128-edge groups
            # (j-major); gather calls take 8 consecutive groups regardless of
            # block boundaries, so every call is a full 1024 indices
            next_call = [0, 0]
            call_tiles = [{}, {}]
            for j in range(NBLK):
                wt = wpool.tile([P, 2 * gh * P], mybir.dt.bfloat16, tag="W")
                nc.sync.dma_start(
                    out=wt[:],
                    in_=w_d.ap()[:, j * 2 * gh * P:(j + 1) * 2 * gh * P])

                for s in range(2):
                    while next_call[s] * 8 < min((j + 1) * gh, nstream):
                        c = next_call[s]
                        g0, g1 = c * 8, min(c * 8 + 8, nstream)
                        cb = s * nstream * 8 + c * 64
                        g = gpool.tile([P, g1 - g0, C], mybir.dt.bfloat16,
                                       tag="gat8" if g1 - g0 == 8 else "gatT")
                        nc.gpsimd.dma_gather(
                            out_ap=g[:],
                            in_ap=src_half_aps[s],
                            idxs_ap=idx_t[:, cb:cb + (g1 - g0) * 8],
                            num_idxs=(g1 - g0) * P,
                            num_idxs_reg=(g1 - g0) * P,
                            elem_size=C,
                            queue_num=qctr % NQ,
                        )
                        qctr += 1
                        call_tiles[s][c] = (g, g0)
                        next_call[s] += 1

                acc = ppool.tile([P, C], mybir.dt.float32, space="PSUM")
                for gi in range(2 * gh):
                    s, gg = divmod(gi, gh)
                    pos = j * gh + gg
                    g, g0 = call_tiles[s][pos // 8]
                    nc.tensor.matmul(
                        out=acc[:],
                        lhsT=wt[:, gi * P:(gi + 1) * P],
                        rhs=g[:, pos - g0, :],
                        start=(gi == 0),
                        stop=(gi == 2 * gh - 1),
                    )
                ot = opool.tile([P, C], mybir.dt.bfloat16, tag="out")
                nc.any.tensor_copy(out=ot[:], in_=acc[:])
                nc.sync.dma_start(out=out_d.ap()[j * P:(j + 1) * P, :], in_=ot[:])

    nc.compile()
    return nc


def kernel(H, edge_index, edge_weight, node_idx):
    H = np.asarray(H)
    edge_index = np.asarray(edge_index)
    edge_weight = np.ascontiguousarray(np.asarray(edge_weight), dtype=np.float32)
    node_idx = np.asarray(node_idx)

    inv = np.argsort(node_idx).astype(np.int64)  # id -> row (identity for arange)

    # ---- host bucketing: (core, dst-block, src-half) ----
    per_core = []   # (sloc_sorted, dloc_sorted, w_sorted, counts) per core
    gh = 1
    for b in range(B):
        dst = inv[edge_index[b, :, 0]]
        src = inv[edge_index[b, :, 1]]
        w = edge_weight[b]
        half = dst >= HALF0
        for h in (0, 1):
            m = half == (h == 1)
            d = dst[m] - h * HALF0
            s_rows = src[m]
            sh = s_rows >= SRC_SPLIT
            sloc = (s_rows - sh * SRC_SPLIT).astype(np.int16)
            bucket = (d >> 7) * 2 + sh
            order = np.argsort(bucket, kind="stable")
            bs = bucket[order]
            counts = np.bincount(bs, minlength=NSEG)
            gh = max(gh, int(np.ceil(counts.max() / P)))
            per_core.append((sloc[order], (d & 127)[order].astype(np.int64),
                             w[m][order], bs, counts))

    ngrp = NSEG * gh
    slots = ngrp * P
    in_maps = []
    for core in range(N_CORES):
        sloc, dloc, wv, bs, counts = per_core[core]
        starts = np.zeros(NSEG, np.int64)
        starts[1:] = np.cumsum(counts)[:-1]
        rank = np.arange(len(bs)) - starts[bs]
        slot = bs.astype(np.int64) * (gh * P) + rank

        sl = np.zeros(slots, np.int16)  # pads gather row 0 with w=0
        dl = np.zeros(slots, np.int64)
        wl = np.zeros(slots, np.float32)
        sl[slot] = sloc
        dl[slot] = dloc
        wl[slot] = wv

        # reorder slots from (j, s, g, e) to two per-half streams (s, j, g, e),
        # then wrap: stream element f -> [f % 16, f // 16], replicated x8
        streams = sl.reshape(NBLK, 2, gh * P).transpose(1, 0, 2).reshape(-1)
        idx16 = streams.reshape(-1, 16).T
        idx128 = np.ascontiguousarray(np.tile(idx16, (8, 1)))

        # one-hot weight tables: group k, edge-slot e (partition), dst col d:
        # W[e, k*128 + d] = w * (d == dloc).  Host-built, streamed per block.
        wtab = np.zeros((slots, P), np.float32)
        wtab[np.arange(slots), dl] = wl
        # reorder to [P partitions, ngrp*P cols]: partition e, col k*128+d
        wtab = np.ascontiguousarray(
            wtab.reshape(ngrp, P, P).transpose(1, 0, 2).reshape(P, ngrp * P)
        ).astype(ml_dtypes.bfloat16)

        in_maps.append({
            "h": np.ascontiguousarray(H[core // 2].reshape(N_NODES, C)).astype(
                ml_dtypes.bfloat16),
            "idx": idx128,
            "w": wtab,
        })

    global _last_in_maps
    _last_in_maps = in_maps
    nc = _prog_cache.get(gh)
    if nc is None:
        nc = _build_program(gh)
        _prog_cache[gh] = nc

    res = run_bass_kernel_spmd(nc, in_maps, list(range(N_CORES)))

    out = np.empty((B, N_NODES, HS, HS), np.float32)
    for b in range(B):
        r0 = res.results[2 * b]["out"].astype(np.float32)
        r1 = res.results[2 * b + 1]["out"].astype(np.float32)
        out[b, :HALF0] = r0.reshape(-1, HS, HS)
        out[b, HALF0:] = r1[:N_NODES - HALF0].reshape(-1, HS, HS)
    return out


# revision 15
# speedup vs baseline: 1.2328x; 1.2328x over previous
"""Neighbor aggregation (GNN message passing) on 8 Trainium2 cores.

out[b, u] = sum_{edges (u, v) in batch b} w_e * H[b, v]    (messages are 16x16 blocks)

Sharding: core (b, h) handles batch b's edges whose destination falls in
dst-half h (h=0: nodes [0, 25088), h=1: [25088, 50048)).  Outputs are disjoint
-> no cross-core reduction.  Within a core, edges are bucketed by 128-node dst
block and by src half (so gather indices fit in int16), padded to a uniform
number of 128-edge groups per bucket.  Device loop per dst block:
  dma_gather 128-row groups of H (bf16, SWDGE) -- calls balanced across the 4
  SWDGE queues so descriptor generation overlaps across Q7 CPU pairs
  one-hot weight matrices W[e, d] = w_e * (d == dstloc_e) are precomputed on
  the host and streamed per block (one dma_start per block, no DVE work)
  PSUM += W.T @ Hgather                                    (bf16 matmul)
  copy PSUM -> SBUF -> DRAM out rows (fp32).
"""

import sys

sys.path.insert(0, "/opt/trn_rl_repo")

import numpy as np
import ml_dtypes

import concourse.bacc as bacc
import concourse.tile as tile
from concourse import mybir
from concourse.bass_utils import run_bass_kernel_spmd

B = 4
N_NODES = 50000
HS = 16
C = HS * HS          # 256 values per message row
P = 128
E = 800000
NBLK = 196           # dst blocks per core (196*128 = 25088 rows of output)
HALF0 = NBLK * P     # dst boundary between the two cores of a batch
SRC_SPLIT = 25000    # src half boundary; local indices stay < 32768 (int16)
NSEG = NBLK * 2      # (block, src-half) buckets per core
N_CORES = 8
NQ = 4               # SWDGE queues (gather descriptor-gen pipelines)

_prog_cache: dict[int, object] = {}
_last_in_maps: list | None = None


def _build_program(gh: int):
    """Bass program for all 8 cores; gh = 128-edge groups per (block, src-half)."""
    ngrp = NSEG * gh             # one-hot groups per core
    idx_cols = NSEG * gh * 8     # int16 idx columns (16 idx per column)

    nc = bacc.Bacc("TRN2", target_bir_lowering=False, debug=False,
                   num_swdge_queues=NQ)
    h_d = nc.dram_tensor("h", (N_NODES, C), mybir.dt.bfloat16, kind="ExternalInput")
    idx_d = nc.dram_tensor("idx", (P, idx_cols), mybir.dt.int16, kind="ExternalInput")
    w_d = nc.dram_tensor("w", (P, ngrp * P), mybir.dt.bfloat16, kind="ExternalInput")
    out_d = nc.dram_tensor("out", (NBLK * P, C), mybir.dt.bfloat16, kind="ExternalOutput")

    h_ap = h_d.ap()
    src_half_aps = (h_ap[0:SRC_SPLIT, :], h_ap[SRC_SPLIT:N_NODES, :])

    nstream = NBLK * gh          # gather groups per src-half stream
    ncall = -(-nstream // 8)     # 1024-idx calls per half
    qctr = 0

    with tile.TileContext(nc) as tc:
        with tc.tile_pool(name="const", bufs=1) as cpool, \
             tc.tile_pool(name="gat", bufs=12) as gpool, \
             tc.tile_pool(name="wtile", bufs=4) as wpool, \
             tc.tile_pool(name="otile", bufs=4) as opool, \
             tc.tile_pool(name="psum", bufs=8, space="PSUM") as ppool:
            idx_t = cpool.tile([P, idx_cols], mybir.dt.int16)
            nc.sync.dma_start(out=idx_t[:], in_=idx_d.ap())

            # each src-half is one contiguous stream of 128-edge groups
            # (j-major); gather calls take 8 consecutive groups regardless of
            # block boundaries, so every call is a full 1024 indices
            next_call = [0, 0]
            call_tiles = [{}, {}]
            for j in range(NBLK):
                wt = wpool.tile([P, 2 * gh * P], mybir.dt.bfloat16, tag="W")
                nc.sync.dma_start(
                    out=wt[:],
                    in_=w_d.ap()[:, j * 2 * gh * P:(j + 1) * 2 * gh * P])

                for s in range(2):
                    while next_call[s] * 8 < min((j + 1) * gh, nstream):
                        c = next_call[s]
                        g0, g1 = c * 8, min(c * 8 + 8, nstream)
                        cb = s * nstream * 8 + c * 64
                        g = gpool.tile([P, g1 - g0, C], mybir.dt.bfloat16,
                                       tag="gat8" if g1 - g0 == 8 else "gatT")
                        nc.gpsimd.dma_gather(
                            out_ap=g[:],
                            in_ap=src_half_aps[s],
                            idxs_ap=idx_t[:, cb:cb + (g1 - g0) * 8],
                            num_idxs=(g1 - g0) * P,
                            num_idxs_reg=(g1 - g0) * P,
                            elem_size=C,
                            queue_num=qctr % NQ,
                        )
                        qctr += 1
                        call_tiles[s][c] = (g, g0)
                        next_call[s] += 1

                acc = ppool.tile([P, C], mybir.dt.float32, space="PSUM")
                for gi in range(2 * gh):
                    s, gg = divmod(gi, gh)
                    pos = j * gh + gg
                    g, g0 = call_tiles[s][pos // 8]
                    nc.tensor.matmul(
                        out=acc[:],
                        lhsT=wt[:, gi * P:(gi + 1) * P],
                        rhs=g[:, pos - g0, :],
                        start=(gi == 0),
                        stop=(gi == 2 * gh - 1),
                    )
                ot = opool.tile([P, C], mybir.dt.bfloat16, tag="out")
                nc.any.tensor_copy(out=ot[:], in_=acc[:])
                nc.sync.dma_start(out=out_d.ap()[j * P:(j + 1) * P, :], in_=ot[:])

    nc.compile()
    return nc


def kernel(H, edge_index, edge_weight, node_idx):
    H = np.asarray(H)
    edge_index = np.asarray(edge_index)
    edge_weight = np.ascontiguousarray(np.asarray(edge_weight), dtype=np.float32)
    node_idx = np.asarray(node_idx)

    inv = np.argsort(node_idx).astype(np.int64)  # id -> row (identity for arange)

    # ---- host bucketing: (core, dst-block, src-half) ----
    per_core = []   # (sloc_sorted, dloc_sorted, w_sorted, counts) per core
    gh = 1
    for b in range(B):
        dst = inv[edge_index[b, :, 0]]
        src = inv[edge_index[b, :, 1]]
        w = edge_weight[b]
        half = dst >= HALF0
        for h in (0, 1):
            m = half == (h == 1)
            d = dst[m] - h * HALF0
            s_rows = src[m]
            sh = s_rows >= SRC_SPLIT
            sloc = (s_rows - sh * SRC_SPLIT).astype(np.int16)
            bucket = (d >> 7) * 2 + sh
            order = np.argsort(bucket, kind="stable")
            bs = bucket[order]
            counts = np.bincount(bs, minlength=NSEG)
            gh = max(gh, int(np.ceil(counts.max() / P)))
            per_core.append((sloc[order], (d & 127)[order].astype(np.int64),
                             w[m][order], bs, counts))

    ngrp = NSEG * gh
    slots = ngrp * P
    in_maps = []
    for core in range(N_CORES):
        sloc, dloc, wv, bs, counts = per_core[core]
        starts = np.zeros(NSEG, np.int64)
        starts[1:] = np.cumsum(counts)[:-1]
        rank = np.arange(len(bs)) - starts[bs]
        slot = bs.astype(np.int64) * (gh * P) + rank

        sl = np.zeros(slots, np.int16)  # pads gather row 0 with w=0
        dl = np.zeros(slots, np.int64)
        wl = np.zeros(slots, np.float32)
        sl[slot] = sloc
        dl[slot] = dloc
        wl[slot] = wv

        # reorder slots from (j, s, g, e) to two per-half streams (s, j, g, e),
        # then wrap: stream element f -> [f % 16, f // 16], replicated x8
        streams = sl.reshape(NBLK, 2, gh * P).transpose(1, 0, 2).reshape(-1)
        idx16 = streams.reshape(-1, 16).T
        idx128 = np.ascontiguousarray(np.tile(idx16, (8, 1)))

        # one-hot weight tables: group k, edge-slot e (partition), dst col d:
        # W[e, k*128 + d] = w * (d == dloc).  Host-built, streamed per block.
        wtab = np.zeros((slots, P), np.float32)
        wtab[np.arange(slots), dl] = wl
        # reorder to [P partitions, ngrp*P cols]: partition e, col k*128+d
        wtab = np.ascontiguousarray(
            wtab.reshape(ngrp, P, P).transpose(1, 0, 2).reshape(P, ngrp * P)
        ).astype(ml_dtypes.bfloat16)

        in_maps.append({
            "h": np.ascontiguousarray(H[core // 2].reshape(N_NODES, C)).astype(
                ml_dtypes.bfloat16),
            "idx": idx128,
            "w": wtab,
        })

    global _last_in_maps
    _last_in_maps = in_maps
    nc = _prog_cache.get(gh)
    if nc is None:
        nc = _build_program(gh)
        _prog_cache[gh] = nc

    res = run_bass_kernel_spmd(nc, in_maps, list(range(N_CORES)))

    out = np.empty((B, N_NODES, HS, HS), np.float32)
    for b in range(B):
        r0 = res.results[2 * b]["out"].astype(np.float32)
        r1 = res.results[2 * b + 1]["out"].astype(np.float32)
        out[b, :HALF0] = r0.reshape(-1, HS, HS)
        out[b, HALF0:] = r1[:N_NODES - HALF0].reshape(-1, HS, HS)
    return out


# revision 17
# speedup vs baseline: 1.3735x; 1.1141x over previous
"""Neighbor aggregation (GNN message passing) on 8 Trainium2 cores.

out[b, u] = sum_{edges (u, v) in batch b} w_e * H[b, v]    (messages are 16x16 blocks)

Sharding: core (b, h) handles batch b's edges whose destination falls in
dst-half h (h=0: nodes [0, 25088), h=1: [25088, 50048)).  Outputs are disjoint
-> no cross-core reduction.  Within a core, edges are bucketed by 128-node dst
block and by src half (so gather indices fit in int16), padded to a uniform
number of 128-edge groups per bucket.  Device loop per dst block:
  dma_gather 128-row groups of H (bf16, SWDGE) -- calls balanced across the 4
  SWDGE queues so descriptor generation overlaps across Q7 CPU pairs
  one-hot weight matrices W[e, d] = w_e * (d == dstloc_e) are precomputed on
  the host and streamed per block (one dma_start per block, no DVE work)
  PSUM += W.T @ Hgather                                    (bf16 matmul)
  copy PSUM -> SBUF -> DRAM out rows (fp32).
"""

import sys

sys.path.insert(0, "/opt/trn_rl_repo")

import numpy as np
import ml_dtypes

import concourse.bacc as bacc
import concourse.tile as tile
from concourse import mybir
from concourse.bass_utils import run_bass_kernel_spmd

B = 4
N_NODES = 50000
HS = 16
C = HS * HS          # 256 values per message row
P = 128
E = 800000
NBLK = 196           # dst blocks per core (196*128 = 25088 rows of output)
HALF0 = NBLK * P     # dst boundary between the two cores of a batch
SRC_SPLIT = 25000    # src half boundary; local indices stay < 32768 (int16)
NSEG = NBLK * 2      # (block, src-half) buckets per core
N_CORES = 8
NQ = 4               # SWDGE queues (gather descriptor-gen pipelines)

# one-hot W windows: within a dloc-sorted bucket, group g's dst offsets
# cluster around their quantile position; store only a window per group.
# PE requires PSUM base partition in {0, 32, 64}.


def _pick_windows(lo, hi):
    """Per-group (start, width) covering [lo_g, hi_g] across all cores."""
    ws, ww = [], []
    for l, h in zip(lo, hi):
        if l > h:          # empty group everywhere
            ws.append(0); ww.append(64); continue
        for s, w in ((0, 64), (64, 64), (0, 96), (0, 128)):
            if s <= l and h < s + w:
                ws.append(s); ww.append(w); break
        else:
            ws.append(0); ww.append(128)
    return tuple(ws), tuple(ww)

_prog_cache: dict[int, object] = {}
_last_in_maps: list | None = None


def _build_program(gh: int, ws: tuple, ww: tuple):
    """Bass program for all 8 cores; gh = 128-edge groups per (block, src-half)."""
    ngrp = NSEG * gh             # one-hot groups per core
    idx_cols = NSEG * gh * 8     # int16 idx columns (16 idx per column)
    wsum = sum(ww)               # W cols per (block, src-half)
    wpre = [0]
    for w in ww[:-1]:
        wpre.append(wpre[-1] + w)

    nc = bacc.Bacc("TRN2", target_bir_lowering=False, debug=False,
                   num_swdge_queues=NQ)
    h_d = nc.dram_tensor("h", (N_NODES, C), mybir.dt.bfloat16, kind="ExternalInput")
    idx_d = nc.dram_tensor("idx", (P, idx_cols), mybir.dt.int16, kind="ExternalInput")
    w_d = nc.dram_tensor("w", (P, NSEG * wsum), mybir.dt.bfloat16, kind="ExternalInput")
    out_d = nc.dram_tensor("out", (NBLK * P, C), mybir.dt.bfloat16, kind="ExternalOutput")

    h_ap = h_d.ap()
    src_half_aps = (h_ap[0:SRC_SPLIT, :], h_ap[SRC_SPLIT:N_NODES, :])

    nstream = NBLK * gh          # gather groups per src-half stream
    ncall = -(-nstream // 8)     # 1024-idx calls per half
    qctr = 0

    with tile.TileContext(nc) as tc:
        with tc.tile_pool(name="const", bufs=1) as cpool, \
             tc.tile_pool(name="gat", bufs=12) as gpool, \
             tc.tile_pool(name="wtile", bufs=4) as wpool, \
             tc.tile_pool(name="otile", bufs=4) as opool, \
             tc.tile_pool(name="psum", bufs=8, space="PSUM") as ppool:
            idx_t = cpool.tile([P, idx_cols], mybir.dt.int16)
            nc.sync.dma_start(out=idx_t[:], in_=idx_d.ap())
            zero_t = cpool.tile([P, C], mybir.dt.bfloat16)
            nc.vector.memset(zero_t[:], 0.0)

            # each src-half is one contiguous stream of 128-edge groups
            # (j-major); gather calls take 8 consecutive groups regardless of
            # block boundaries, so every call is a full 1024 indices
            next_call = [0, 0]
            call_tiles = [{}, {}]
            for j in range(NBLK):
                wt = wpool.tile([P, 2 * wsum], mybir.dt.bfloat16, tag="W")
                nc.sync.dma_start(
                    out=wt[:],
                    in_=w_d.ap()[:, j * 2 * wsum:(j + 1) * 2 * wsum])

                for s in range(2):
                    while next_call[s] * 8 < min((j + 1) * gh, nstream):
                        c = next_call[s]
                        g0, g1 = c * 8, min(c * 8 + 8, nstream)
                        cb = s * nstream * 8 + c * 64
                        g = gpool.tile([P, g1 - g0, C], mybir.dt.bfloat16,
                                       tag="gat8" if g1 - g0 == 8 else "gatT")
                        nc.gpsimd.dma_gather(
                            out_ap=g[:],
                            in_ap=src_half_aps[s],
                            idxs_ap=idx_t[:, cb:cb + (g1 - g0) * 8],
                            num_idxs=(g1 - g0) * P,
                            num_idxs_reg=(g1 - g0) * P,
                            elem_size=C,
                            queue_num=qctr % NQ,
                        )
                        qctr += 1
                        call_tiles[s][c] = (g, g0)
                        next_call[s] += 1

                acc = ppool.tile([P, C], mybir.dt.float32, space="PSUM")
                # zero the full accumulator with a zero-weights matmul, then
                # accumulate each group into its 64/96-wide dst window
                nc.tensor.matmul(
                    out=acc[:],
                    lhsT=zero_t[:, 0:P],
                    rhs=zero_t[:],
                    start=True,
                    stop=False,
                    skip_group_check=True,
                )
                for gi in range(2 * gh):
                    s, gg = divmod(gi, gh)
                    pos = j * gh + gg
                    g, g0 = call_tiles[s][pos // 8]
                    off = s * wsum + wpre[gg]
                    nc.tensor.matmul(
                        out=acc[ws[gg]:ws[gg] + ww[gg], :],
                        lhsT=wt[:, off:off + ww[gg]],
                        rhs=g[:, pos - g0, :],
                        start=False,
                        stop=(gi == 2 * gh - 1),
                        skip_group_check=True,
                    )
                ot = opool.tile([P, C], mybir.dt.bfloat16, tag="out")
                nc.any.tensor_copy(out=ot[:], in_=acc[:])
                nc.sync.dma_start(out=out_d.ap()[j * P:(j + 1) * P, :], in_=ot[:])

    nc.compile()
    return nc


def kernel(H, edge_index, edge_weight, node_idx):
    H = np.asarray(H)
    edge_index = np.asarray(edge_index)
    edge_weight = np.ascontiguousarray(np.asarray(edge_weight), dtype=np.float32)
    node_idx = np.asarray(node_idx)

    inv = np.argsort(node_idx).astype(np.int64)  # id -> row (identity for arange)

    # ---- host bucketing: (core, dst-block, src-half) ----
    per_core = []   # (sloc_sorted, dloc_sorted, w_sorted, counts) per core
    gh = 1
    for b in range(B):
        dst = inv[edge_index[b, :, 0]]
        src = inv[edge_index[b, :, 1]]
        w = edge_weight[b]
        half = dst >= HALF0
        for h in (0, 1):
            m = half == (h == 1)
            d = dst[m] - h * HALF0
            s_rows = src[m]
            sh = s_rows >= SRC_SPLIT
            sloc = (s_rows - sh * SRC_SPLIT).astype(np.int16)
            bucket = (d >> 7) * 2 + sh
            order = np.lexsort(((d & 127), bucket))
            bs = bucket[order]
            counts = np.bincount(bs, minlength=NSEG)
            gh = max(gh, int(np.ceil(counts.max() / P)))
            per_core.append((sloc[order], (d & 127)[order].astype(np.int64),
                             w[m][order], bs, counts))

    ngrp = NSEG * gh
    slots = ngrp * P

    # per-group dloc ranges across all cores -> shared windows
    glo = np.full(gh, 128, np.int64)
    ghi = np.full(gh, -1, np.int64)
    for core in range(N_CORES):
        sloc, dloc, wv, bs, counts = per_core[core]
        starts = np.zeros(NSEG, np.int64)
        starts[1:] = np.cumsum(counts)[:-1]
        rank = np.arange(len(bs)) - starts[bs]
        gidx = rank // P
        np.minimum.at(glo, gidx, dloc)
        np.maximum.at(ghi, gidx, dloc)
    ws, ww = _pick_windows(glo, ghi)
    wsum = sum(ww)
    wpre = np.zeros(gh, np.int64)
    wpre[1:] = np.cumsum(ww)[:-1]

    in_maps = []
    for core in range(N_CORES):
        sloc, dloc, wv, bs, counts = per_core[core]
        starts = np.zeros(NSEG, np.int64)
        starts[1:] = np.cumsum(counts)[:-1]
        rank = np.arange(len(bs)) - starts[bs]
        slot = bs.astype(np.int64) * (gh * P) + rank

        sl = np.zeros(slots, np.int16)  # pads gather row 0 with w=0
        dl = np.zeros(slots, np.int64)
        wl = np.zeros(slots, np.float32)
        sl[slot] = sloc
        dl[slot] = dloc
        wl[slot] = wv

        # reorder slots from (j, s, g, e) to two per-half streams (s, j, g, e),
        # then wrap: stream element f -> [f % 16, f // 16], replicated x8
        streams = sl.reshape(NBLK, 2, gh * P).transpose(1, 0, 2).reshape(-1)
        idx16 = streams.reshape(-1, 16).T
        idx128 = np.ascontiguousarray(np.tile(idx16, (8, 1)))

        # variable-width windowed one-hot tables, layout per (block, half):
        # cols [seg*wsum + wpre[g], +ww[g])
        k = np.arange(slots)
        seg = k // (gh * P)
        glocal = (k // P) % gh
        erow = k % P
        ws_arr = np.asarray(ws, np.int64)[glocal]
        wcol = np.clip(dl - ws_arr, 0, np.asarray(ww, np.int64)[glocal] - 1)
        col = seg * wsum + wpre[glocal] + wcol
        wtab = np.zeros((P, NSEG * wsum), np.float32)
        wtab[erow, col] = wl
        wtab = np.ascontiguousarray(wtab).astype(ml_dtypes.bfloat16)

        in_maps.append({
            "h": np.ascontiguousarray(H[core // 2].reshape(N_NODES, C)).astype(
                ml_dtypes.bfloat16),
            "idx": idx128,
            "w": wtab,
        })

    global _last_in_maps
    _last_in_maps = in_maps
    key = (gh, ws, ww)
    nc = _prog_cache.get(key)
    if nc is None:
        nc = _build_program(gh, ws, ww)
        _prog_cache[key] = nc

    res = run_bass_kernel_spmd(nc, in_maps, list(range(N_CORES)))

    out = np.empty((B, N_NODES, HS, HS), np.float32)
    for b in range(B):
        r0 = res.results[2 * b]["out"].astype(np.float32)
        r1 = res.results[2 * b + 1]["out"].astype(np.float32)
        out[b, :HALF0] = r0.reshape(-1, HS, HS)
        out[b, HALF0:] = r1[:N_NODES - HALF0].reshape(-1, HS, HS)
    return out
